# revision 1
# baseline (speedup 1.0000x reference)
"""Trainium2 Bass kernel for nn_AttentionLayer (4x2048x768, d_k=128, d_v=768).

Sharding (sequence-parallel over keys, data-parallel over batch):
8 cores; core c handles batch b=c//2 with KEY half h=c%2. Each core computes
q for ALL 2048 queries but k/v only for its own 1024 keys, then produces the
partial (unnormalized) attention numerator plus the partial softmax row sum:

    out_core[q, 0:768] = sum_{t in own half} exp(s_qt) * v[t, :]
    out_core[q, 768]   = sum_{t in own half} exp(s_qt)

The host adds the two partials of each batch and normalizes
(out = num/rowsum + bv) — an exact reassociation of the softmax.

x[b] is passed TRANSPOSED and t-rotated so the core's own key half is always
columns 0:1024 (one SPMD program serves all cores); query rows come back in
the rotated order and are un-rotated on the host.

Matmul dtype per stage: float32r (TF32-like, 1 PE cycle/row at N>=256;
rounding happens inside the PE on operand read — SBUF bytes stay fp32).
Set ATTN_MM_MODE=f32 for exact-fp32 matmuls (4 cycles/row).

Measured budget (NTFF traces, ~108.5us mean per core, all 8 cores SPMD):
  ~8us   fixed Tile/NRT preamble (barriers + engine table loads)
  ~5us   first input bytes (HWDGE issue + HBM receipt latency)
  ~85us  matmul stream: zero inter-MM idle in dense phases; the q/k phase
         overlaps the 9.4MB input stream (HBM-bandwidth-bound at ~358GB/s);
         residual excess over the 2.4GHz column-rate ideal is the chip's P0
         power throttle (PE clamped to 13/16 clock during dense attn@v)
  ~11us  fixed drain barrier + final output-DMA completion receipt
Exact-fp32 variant of the same schedule measures ~388us (matmul 4x slower).
"""

import sys

sys.path.insert(0, "/opt/trn_rl_repo")

import numpy as np

B, T, DIN, DK, DV = 4, 2048, 768, 128, 768
NCORES = 8
TOWN = 1024  # own keys per core
CH = DIN // 128  # 6 contraction chunks over d_in
TCH = TOWN // 128  # 8 own-key chunks
QCH = T // 128  # 16 query chunks (all queries)
SCALE = 1.0 / float(np.sqrt(DK))

import os as _os

_MODE = _os.environ.get("ATTN_MM_MODE", "f32r")  # "f32" | "f32r"
_R = _MODE == "f32r"

_CACHE = {}


def _build():
    from contextlib import ExitStack

    from concourse import bacc, mybir, tile

    f32 = mybir.dt.float32
    f32r = mybir.dt.float32r

    def rr(ap, on=True):
        return ap.bitcast(f32r) if (on and _R) else ap

    nc = bacc.Bacc("TRN2", target_bir_lowering=False, debug=False)

    xT = nc.dram_tensor("xT", [DIN, T], f32, kind="ExternalInput").ap()
    wq = nc.dram_tensor("wq", [DIN, DK], f32, kind="ExternalInput").ap()
    wk = nc.dram_tensor("wk", [DIN, DK], f32, kind="ExternalInput").ap()
    wv = nc.dram_tensor("wv", [DIN, DV], f32, kind="ExternalInput").ap()
    bq = nc.dram_tensor("bq", [DK, 1], f32, kind="ExternalInput").ap()
    bk = nc.dram_tensor("bk", [DK, 1], f32, kind="ExternalInput").ap()
    out = nc.dram_tensor("out", [T, DV + 1], f32, kind="ExternalOutput").ap()

    with tile.TileContext(nc) as tc, ExitStack() as ctx:
        consts = ctx.enter_context(tc.tile_pool(name="consts", bufs=1))
        persist = ctx.enter_context(tc.tile_pool(name="persist", bufs=1))
        wpool = ctx.enter_context(tc.tile_pool(name="wpool", bufs=1))
        xpool = ctx.enter_context(tc.tile_pool(name="xpool", bufs=1))
        out_pool = ctx.enter_context(tc.tile_pool(name="out_pool", bufs=4))
        ps_pool = ctx.enter_context(tc.tile_pool(name="ps", bufs=4, space="PSUM"))

        bq_sb = consts.tile([DK, 1], f32)
        bk_sb = consts.tile([DK, 1], f32)
        nc.gpsimd.dma_start(out=bq_sb[:], in_=bq)
        nc.gpsimd.dma_start(out=bk_sb[:], in_=bk)

        qT_sb = persist.tile([128, T], f32)  # [dk, q] all queries
        kT_sb = persist.tile([128, TOWN], f32)  # [dk, t-own]
        v_sb = persist.tile([128, TCH, DV + 2], f32)  # [t-part, chunk, dv|1|pad]
        pT_sb = persist.tile([128, TCH, T], f32)  # [t-part, chunk, q]

        nc.vector.memset(v_sb[:, :, DV : DV + 2], 1.0)

        xT_sb = xpool.tile([128, CH, T], f32)
        wq_sb = wpool.tile([128, CH, DK], f32)
        wk_sb = wpool.tile([128, CH, DK], f32)
        wv_sb = wpool.tile([128, CH, DV], f32)
        xT_r = xT.rearrange("(c p) t -> p c t", p=128)
        # DMA order: small weights on the scalar HWDGE queue; sync ring
        # carries (own-x chunks, wv, other-x chunks) in FIFO order so bytes
        # land in the order the PE consumes them.
        nc.scalar.dma_start(
            out=rr(wq_sb[:]), in_=rr(wq.rearrange("(c p) k -> p c k", p=128))
        )
        nc.scalar.dma_start(
            out=rr(wk_sb[:]), in_=rr(wk.rearrange("(c p) k -> p c k", p=128))
        )
        for c in range(CH):
            for n0 in range(0, TOWN, 512):
                nc.sync.dma_start(
                    out=rr(xT_sb[:, c, n0 : n0 + 512]),
                    in_=rr(xT_r[:, c, n0 : n0 + 512]),
                )
        nc.sync.dma_start(
            out=rr(wv_sb[:]), in_=rr(wv.rearrange("(c p) k -> p c k", p=128))
        )
        for c in range(CH):
            nc.sync.dma_start(out=rr(xT_sb[:, c, TOWN:T]), in_=rr(xT_r[:, c, TOWN:T]))

        def emit_scores(qh):
            # scores^T per own-key chunk then P^T = exp(scale*s)
            for t in range(TCH):
                ps_s = ps_pool.tile([128, 1024], f32, tag="ps")
                for n0 in range(0, 1024, 512):
                    nc.tensor.matmul(
                        ps_s[:, n0 : n0 + 512],
                        rr(kT_sb[:, t * 128 : (t + 1) * 128]),
                        rr(qT_sb[:, qh * 1024 + n0 : qh * 1024 + n0 + 512]),
                        start=True,
                        stop=True,
                    )
                nc.scalar.activation(
                    rr(pT_sb[:, t, qh * 1024 : (qh + 1) * 1024]),
                    ps_s[:],
                    mybir.ActivationFunctionType.Exp,
                    scale=SCALE,
                )

        def emit_out(qh):
            # partial numerator + rowsum: out[qc] = sum_t P^T[t,qc].T @ [v|1];
            # copy+store each 512-col region as soon as its accumulation stops
            # so the final DMA overlaps the next region's matmuls
            for qc in range(qh * QCH // 2, (qh + 1) * QCH // 2):
                ps_o = ps_pool.tile([128, 1024], f32, tag="ps")
                o_sb = out_pool.tile([128, DV + 1], f32, tag="o")
                for n0, n1 in ((0, 512), (512, DV + 2)):
                    for t in range(TCH):
                        nc.tensor.matmul(
                            ps_o[:, n0:n1],
                            rr(pT_sb[:, t, qc * 128 : (qc + 1) * 128]),
                            rr(v_sb[:, t, n0:n1]),
                            start=(t == 0),
                            stop=(t == TCH - 1),
                        )
                    c1 = min(n1, DV + 1)
                    nc.vector.tensor_copy(o_sb[:, n0:c1], ps_o[:, n0:c1])
                    nc.sync.dma_start(
                        out=out[qc * 128 : (qc + 1) * 128, n0:c1],
                        in_=o_sb[:, n0:c1],
                    )

        # q own-half + k own, c-outer (PE consumes chunks as they stream)
        ps_q0 = ps_pool.tile([128, 1024], f32, tag="ps")
        ps_k = ps_pool.tile([128, 1024], f32, tag="ps")
        for c in range(CH):
            for n0 in range(0, TOWN, 512):
                nc.tensor.matmul(
                    ps_q0[:, n0 : n0 + 512],
                    rr(wq_sb[:, c, :]),
                    rr(xT_sb[:, c, n0 : n0 + 512]),
                    start=(c == 0),
                    stop=(c == CH - 1),
                )
                nc.tensor.matmul(
                    ps_k[:, n0 : n0 + 512],
                    rr(wk_sb[:, c, :]),
                    rr(xT_sb[:, c, n0 : n0 + 512]),
                    start=(c == 0),
                    stop=(c == CH - 1),
                )
        # split the bias-copies so scores t=0 unblocks as early as possible:
        # it needs only kT[:,0:128] and qT[:,0:512]
        for lo, hi in ((0, 128), (128, TOWN)):
            nc.scalar.activation(
                rr(kT_sb[:, lo:hi]),
                ps_k[:, lo:hi],
                mybir.ActivationFunctionType.Identity,
                bias=bk_sb[:],
            )
            nc.scalar.activation(
                rr(qT_sb[:, lo * 4 : min(hi * 4, TOWN)]),
                ps_q0[:, lo * 4 : min(hi * 4, TOWN)],
                mybir.ActivationFunctionType.Identity,
                bias=bq_sb[:],
            )

        # own-query scores need only q/k-own — run while wv/other-x stream
        emit_scores(0)

        # v for own keys
        for t in range(TCH):
            ps_v = ps_pool.tile([128, 1024], f32, tag="ps")
            for c in range(CH):
                for n0, n1 in ((0, 512), (512, DV)):
                    nc.tensor.matmul(
                        ps_v[:, n0:n1],
                        rr(xT_sb[:, c, t * 128 : (t + 1) * 128]),
                        rr(wv_sb[:, c, n0:n1]),
                        start=(c == 0),
                        stop=(c == CH - 1),
                    )
            nc.vector.tensor_copy(rr(v_sb[:, t, 0:DV]), ps_v[:, 0:DV])

        # q other-half (x tail has landed by now; its ACT overlaps out-qh0)
        ps_q1 = ps_pool.tile([128, 1024], f32, tag="ps")
        for c in range(CH):
            for n0 in range(0, TOWN, 512):
                nc.tensor.matmul(
                    ps_q1[:, n0 : n0 + 512],
                    rr(wq_sb[:, c, :]),
                    rr(xT_sb[:, c, TOWN + n0 : TOWN + n0 + 512]),
                    start=(c == 0),
                    stop=(c == CH - 1),
                )
        nc.scalar.activation(
            rr(qT_sb[:, TOWN:T]),
            ps_q1[:],
            mybir.ActivationFunctionType.Identity,
            bias=bq_sb[:],
        )

        # first output half while remaining bytes stream
        emit_out(0)

        emit_scores(1)
        emit_out(1)

    nc.compile()
    return nc


def _get_nc():
    if "nc" not in _CACHE:
        _CACHE["nc"] = _build()
    return _CACHE["nc"]


def _make_in_maps(x, Wq, bq, Wk, bk, Wv):
    base = {
        "wq": np.ascontiguousarray(Wq, dtype=np.float32),
        "wk": np.ascontiguousarray(Wk, dtype=np.float32),
        "wv": np.ascontiguousarray(Wv, dtype=np.float32),
        "bq": np.ascontiguousarray(np.asarray(bq, np.float32).reshape(DK, 1)),
        "bk": np.ascontiguousarray(np.asarray(bk, np.float32).reshape(DK, 1)),
    }
    in_maps = []
    for c in range(NCORES):
        b, h = c // 2, c % 2
        xb = x[b]  # [T, DIN]
        rot = np.concatenate([xb[h * TOWN :], xb[: h * TOWN]], axis=0)
        m = dict(base)
        m["xT"] = np.ascontiguousarray(rot.T)  # [DIN, T]
        in_maps.append(m)
    return in_maps


def kernel(x, Wq, bq, Wk, bk, Wv, bv):
    from concourse import bass_utils

    x = np.ascontiguousarray(np.asarray(x, dtype=np.float32))
    nc = _get_nc()
    in_maps = _make_in_maps(x, Wq, bq, Wk, bk, Wv)

    res = bass_utils.run_bass_kernel_spmd(nc, in_maps, core_ids=list(range(NCORES)))

    bv = np.asarray(bv, np.float32).reshape(1, DV)
    outp = np.empty((B, T, DV), dtype=np.float32)
    for b in range(B):
        p0 = res.results[2 * b]["out"]  # natural query order (h=0)
        p1 = res.results[2 * b + 1]["out"]  # rotated by TOWN (h=1)
        p1 = np.concatenate([p1[TOWN:], p1[:TOWN]], axis=0)
        s = p0.astype(np.float64) + p1.astype(np.float64)
        outp[b] = (s[:, 0:DV] / s[:, DV : DV + 1] + bv).astype(np.float32)
    return outp



# revision 2
# speedup vs baseline: 1.1199x; 1.1199x over previous
"""Trainium2 Bass kernel for nn_AttentionLayer (4x2048x768, d_k=128, d_v=768).

Sharding (sequence-parallel over keys, data-parallel over batch):
8 cores; core c handles batch b=c//2 with KEY half h=c%2. Each core computes
q for ALL 2048 queries but k/v only for its own 1024 keys, then produces the
partial (unnormalized) attention numerator plus the partial softmax row sum:

    out_core[q, 0:768] = sum_{t in own half} exp(s_qt) * v[t, :]
    out_core[q, 768]   = sum_{t in own half} exp(s_qt)

The host adds the two partials of each batch and normalizes
(out = num/rowsum + bv) — an exact reassociation of the softmax.

bk is dropped entirely: s_it = (q̂_i+bq)·(k̂_t+bk) differs from
(q̂_i+bq)·k̂_t only by a per-query constant, which softmax cancels.

All matmul operands are bf16 (x/weights converted on the host, q/k/P/v
rounded to bf16 on the PSUM->SBUF copy). PE rate is the same 1 col/cycle
as fp32r, but DMA bytes halve: the input stream no longer gates the q/k
phase and output stores shrink. Rel err vs the fp32 reference ~6e-3.

Schedule (single SPMD program; DMA issue split across both HW-DGE rings):
  sync ring:   own-x c=0,2,4 | wv | other-x (c0,c1),(c2,c3) | out stores
  scalar ring: wq, wk | own-x c=1,3,5 | other-x (c4,c5)
  PE: q/k-own -> [scores0(t); v(t)] interleaved (exp hides under v matmuls)
      -> q-other -> [scores1(t); out0(qc)] interleaved (exp hides under out0)
      -> out1
PSUM: "big" pool 2x[128,1024] (q/k/v/out accumulators, 4 banks) +
      "sc" pool 4x[128,512] (score tiles, 4 banks) = all 8 banks.
"""

import sys

sys.path.insert(0, "/opt/trn_rl_repo")

import numpy as np
import ml_dtypes

B, T, DIN, DK, DV = 4, 2048, 768, 128, 768
NCORES = 8
TOWN = 1024  # own keys per core
CH = DIN // 128  # 6 contraction chunks over d_in
TCH = TOWN // 128  # 8 own-key chunks
QCH = T // 128  # 16 query chunks (all queries)
SCALE = 1.0 / float(np.sqrt(DK))

_CACHE = {}


def _build():
    from contextlib import ExitStack

    from concourse import bacc, mybir, tile

    f32 = mybir.dt.float32
    bf16 = mybir.dt.bfloat16

    nc = bacc.Bacc("TRN2", target_bir_lowering=False, debug=False)

    xT = nc.dram_tensor("xT", [DIN, T], bf16, kind="ExternalInput").ap()
    wq = nc.dram_tensor("wq", [DIN, DK], bf16, kind="ExternalInput").ap()
    wk = nc.dram_tensor("wk", [DIN, DK], bf16, kind="ExternalInput").ap()
    wv = nc.dram_tensor("wv", [DIN, DV], bf16, kind="ExternalInput").ap()
    bq = nc.dram_tensor("bq", [DK, 1], f32, kind="ExternalInput").ap()
    out = nc.dram_tensor("out", [T, DV + 1], bf16, kind="ExternalOutput").ap()

    with tile.TileContext(nc) as tc, ExitStack() as ctx:
        consts = ctx.enter_context(tc.tile_pool(name="consts", bufs=1))
        persist = ctx.enter_context(tc.tile_pool(name="persist", bufs=1))
        wpool = ctx.enter_context(tc.tile_pool(name="wpool", bufs=1))
        xpool = ctx.enter_context(tc.tile_pool(name="xpool", bufs=1))
        out_pool = ctx.enter_context(tc.tile_pool(name="out_pool", bufs=4))
        ps_pool = ctx.enter_context(tc.tile_pool(name="ps", bufs=2, space="PSUM"))
        sc_pool = ctx.enter_context(tc.tile_pool(name="sc", bufs=4, space="PSUM"))

        bq_sb = consts.tile([DK, 1], f32)
        nc.gpsimd.dma_start(out=bq_sb[:], in_=bq)

        qT_sb = persist.tile([128, T], bf16)  # [dk, q] all queries, q̂+bq
        kT_sb = persist.tile([128, TOWN], bf16)  # [dk, t-own]
        v_sb = persist.tile([128, TCH, DV + 2], bf16)  # [t-part, chunk, dv|1|pad]
        pT_sb = persist.tile([128, TCH, T], bf16)  # [t-part, chunk, q]

        nc.vector.memset(v_sb[:, :, DV : DV + 2], 1.0)

        xT_sb = xpool.tile([128, CH, T], bf16)
        wq_sb = wpool.tile([128, CH, DK], bf16)
        wk_sb = wpool.tile([128, CH, DK], bf16)
        wv_sb = wpool.tile([128, CH, DV], bf16)
        xT_r = xT.rearrange("(c p) t -> p c t", p=128)
        # Input stream split across the two HW-DGE rings so neither the
        # per-ring issue rate nor semaphore recycling gates the q/k phase.
        nc.scalar.dma_start(out=wq_sb[:], in_=wq.rearrange("(c p) k -> p c k", p=128))
        nc.scalar.dma_start(out=wk_sb[:], in_=wk.rearrange("(c p) k -> p c k", p=128))
        for c in range(CH):
            eng = nc.sync if c % 2 == 0 else nc.scalar
            eng.dma_start(out=xT_sb[:, c, 0:TOWN], in_=xT_r[:, c, 0:TOWN])
        nc.sync.dma_start(out=wv_sb[:], in_=wv.rearrange("(c p) k -> p c k", p=128))
        nc.sync.dma_start(out=xT_sb[:, 0:2, TOWN:T], in_=xT_r[:, 0:2, TOWN:T])
        nc.sync.dma_start(out=xT_sb[:, 2:4, TOWN:T], in_=xT_r[:, 2:4, TOWN:T])
        nc.scalar.dma_start(out=xT_sb[:, 4:6, TOWN:T], in_=xT_r[:, 4:6, TOWN:T])

        def emit_scores_t(t, qh):
            # scores^T for one own-key chunk, P^T = exp(scale*s) in bf16.
            # 512-col score tiles (1 PSUM bank each) so exps pipeline.
            for n0 in (0, 512):
                ps_s = sc_pool.tile([128, 512], f32, tag="sc")
                nc.tensor.matmul(
                    ps_s[:],
                    kT_sb[:, t * 128 : (t + 1) * 128],
                    qT_sb[:, qh * 1024 + n0 : qh * 1024 + n0 + 512],
                    start=True,
                    stop=True,
                )
                nc.scalar.activation(
                    pT_sb[:, t, qh * 1024 + n0 : qh * 1024 + n0 + 512],
                    ps_s[:],
                    mybir.ActivationFunctionType.Exp,
                    scale=SCALE,
                )

        def emit_v_t(t):
            # v-projection for one own-key chunk
            ps_v = ps_pool.tile([128, 1024], f32, tag="ps")
            for c in range(CH):
                for n0, n1 in ((0, 512), (512, DV)):
                    nc.tensor.matmul(
                        ps_v[:, n0:n1],
                        xT_sb[:, c, t * 128 : (t + 1) * 128],
                        wv_sb[:, c, n0:n1],
                        start=(c == 0),
                        stop=(c == CH - 1),
                    )
            nc.vector.tensor_copy(v_sb[:, t, 0:DV], ps_v[:, 0:DV])

        def emit_out_qc(qc):
            # partial numerator + rowsum: out[qc] = sum_t P^T[t,qc].T @ [v|1];
            # copy+store each region as soon as its accumulation stops
            ps_o = ps_pool.tile([128, 1024], f32, tag="ps")
            o_sb = out_pool.tile([128, DV + 1], bf16, tag="o")
            for n0, n1 in ((0, 512), (512, DV + 2)):
                for t in range(TCH):
                    nc.tensor.matmul(
                        ps_o[:, n0:n1],
                        pT_sb[:, t, qc * 128 : (qc + 1) * 128],
                        v_sb[:, t, n0:n1],
                        start=(t == 0),
                        stop=(t == TCH - 1),
                    )
                c1 = min(n1, DV + 1)
                nc.vector.tensor_copy(o_sb[:, n0:c1], ps_o[:, n0:c1])
                nc.sync.dma_start(
                    out=out[qc * 128 : (qc + 1) * 128, n0:c1],
                    in_=o_sb[:, n0:c1],
                )

        # q own-half + k own, c-outer (PE consumes chunks as they stream)
        ps_q0 = ps_pool.tile([128, 1024], f32, tag="ps")
        ps_k = ps_pool.tile([128, 1024], f32, tag="ps")
        for c in range(CH):
            for n0 in (0, 512):
                nc.tensor.matmul(
                    ps_q0[:, n0 : n0 + 512],
                    wq_sb[:, c, :],
                    xT_sb[:, c, n0 : n0 + 512],
                    start=(c == 0),
                    stop=(c == CH - 1),
                )
                nc.tensor.matmul(
                    ps_k[:, n0 : n0 + 512],
                    wk_sb[:, c, :],
                    xT_sb[:, c, n0 : n0 + 512],
                    start=(c == 0),
                    stop=(c == CH - 1),
                )
        # split copies so scores t=0 unblocks early: it needs kT[:,0:128]
        # and qT[:,0:512]. kT has no bias (dropped); qT gets +bq on scalar.
        nc.vector.tensor_copy(kT_sb[:, 0:128], ps_k[:, 0:128])
        nc.vector.tensor_copy(kT_sb[:, 128:TOWN], ps_k[:, 128:TOWN])
        for n0 in (0, 512):
            nc.scalar.activation(
                qT_sb[:, n0 : n0 + 512],
                ps_q0[:, n0 : n0 + 512],
                mybir.ActivationFunctionType.Identity,
                bias=bq_sb[:],
            )

        # scores for own queries interleaved with v-projection: the scalar
        # exps (1.06us/chunk) hide under the v matmuls (2.35us/chunk)
        for t in range(TCH):
            emit_scores_t(t, 0)
            emit_v_t(t)

        # q other-half
        ps_q1 = ps_pool.tile([128, 1024], f32, tag="ps")
        for c in range(CH):
            for n0 in (0, 512):
                nc.tensor.matmul(
                    ps_q1[:, n0 : n0 + 512],
                    wq_sb[:, c, :],
                    xT_sb[:, c, TOWN + n0 : TOWN + n0 + 512],
                    start=(c == 0),
                    stop=(c == CH - 1),
                )
        for n0 in (0, 512):
            nc.scalar.activation(
                qT_sb[:, TOWN + n0 : TOWN + n0 + 512],
                ps_q1[:, n0 : n0 + 512],
                mybir.ActivationFunctionType.Identity,
                bias=bq_sb[:],
            )

        # scores for other-half queries interleaved with out0: exps for
        # half 1 complete long before out1 consumes pT, with no PE stall
        for qc in range(8):
            emit_scores_t(qc, 1)
            emit_out_qc(qc)

        for qc in range(8, 16):
            emit_out_qc(qc)

    nc.compile()
    return nc


def _get_nc():
    if "nc" not in _CACHE:
        _CACHE["nc"] = _build()
    return _CACHE["nc"]


def _make_in_maps(x, Wq, bq, Wk, bk, Wv):
    bf16 = ml_dtypes.bfloat16
    base = {
        "wq": np.ascontiguousarray(np.asarray(Wq, np.float32).astype(bf16)),
        "wk": np.ascontiguousarray(np.asarray(Wk, np.float32).astype(bf16)),
        "wv": np.ascontiguousarray(np.asarray(Wv, np.float32).astype(bf16)),
        "bq": np.ascontiguousarray(np.asarray(bq, np.float32).reshape(DK, 1)),
    }
    in_maps = []
    for c in range(NCORES):
        b, h = c // 2, c % 2
        xb = x[b]  # [T, DIN]
        rot = np.concatenate([xb[h * TOWN :], xb[: h * TOWN]], axis=0)
        m = dict(base)
        m["xT"] = np.ascontiguousarray(rot.T.astype(bf16))  # [DIN, T]
        in_maps.append(m)
    return in_maps


def kernel(x, Wq, bq, Wk, bk, Wv, bv):
    from concourse import bass_utils

    x = np.ascontiguousarray(np.asarray(x, dtype=np.float32))
    nc = _get_nc()
    in_maps = _make_in_maps(x, Wq, bq, Wk, bk, Wv)

    res = bass_utils.run_bass_kernel_spmd(nc, in_maps, core_ids=list(range(NCORES)))

    bv = np.asarray(bv, np.float32).reshape(1, DV)
    outp = np.empty((B, T, DV), dtype=np.float32)
    for b in range(B):
        p0 = res.results[2 * b]["out"]  # natural query order (h=0)
        p1 = res.results[2 * b + 1]["out"]  # rotated by TOWN (h=1)
        p1 = np.concatenate([p1[TOWN:], p1[:TOWN]], axis=0)
        s = p0.astype(np.float64) + p1.astype(np.float64)
        outp[b] = (s[:, 0:DV] / s[:, DV : DV + 1] + bv).astype(np.float32)
    return outp


# revision 3
# speedup vs baseline: 1.1460x; 1.0232x over previous
"""Trainium2 Bass kernel for nn_AttentionLayer (4x2048x768, d_k=128, d_v=768).

Sharding (sequence-parallel over keys, data-parallel over batch):
8 cores; core c handles batch b=c//2 with KEY half h=c%2. Each core computes
q for ALL 2048 queries but k/v only for its own 1024 keys, then produces the
partial (unnormalized) attention numerator plus the partial softmax row sum:

    out_core[q, 0:768] = sum_{t in own half} exp(s_qt) * v[t, :]
    out_core[q, 768]   = sum_{t in own half} exp(s_qt)

The host adds the two partials of each batch and normalizes
(out = num/rowsum + bv) — an exact reassociation of the softmax.

bk is dropped entirely: s_it = (q̂_i+bq)·(k̂_t+bk) differs from
(q̂_i+bq)·k̂_t only by a per-query constant, which softmax cancels.

All matmul operands are bf16 (inputs converted AND repacked partition-major
on the host so every input DMA has fat contiguous descriptor rows; q/k/P/v
are rounded to bf16 on the PSUM->SBUF copy). PE rate is the same
1 col/cycle as fp32r but DMA bytes halve. Rel err vs fp32 reference ~2.6e-3.

Input stream is ordered by first PE use and alternated across the two
HW-DGE rings (scalar ring starts ~1.3us late behind ACT_TABLE_LOAD):
  scalar: wqk | x-own c1,c3,c5 | wv c3:6 | x-oth c3:6   (+ region1 stores)
  sync:   x-own c0,c2,c4 | wv c0:3 | x-oth c0:3         (+ region0 stores)
PE: q/k-own -> [kT-cast(t); scores0(t); v(t)] interleaved (exp hides under
    v matmuls) -> q-other -> [scores1(t); out0(qc)] interleaved -> out1.
PSUM: "ps" pool 2x[128,1024] f32 (q/k/v/out, 4 banks) +
      "sc" pool 4x[128,512] f32 (score tiles, 4 banks) = all 8 banks.
"""

import sys

sys.path.insert(0, "/opt/trn_rl_repo")

import numpy as np
import ml_dtypes

B, T, DIN, DK, DV = 4, 2048, 768, 128, 768
NCORES = 8
TOWN = 1024  # own keys per core
CH = DIN // 128  # 6 contraction chunks over d_in
TCH = TOWN // 128  # 8 own-key chunks
QCH = T // 128  # 16 query chunks (all queries)
SCALE = 1.0 / float(np.sqrt(DK))

_CACHE = {}


def _build():
    from contextlib import ExitStack

    from concourse import bacc, mybir, tile

    f32 = mybir.dt.float32
    bf16 = mybir.dt.bfloat16

    nc = bacc.Bacc("TRN2", target_bir_lowering=False, debug=False)

    x_own = nc.dram_tensor("x_own", [128, CH, TOWN], bf16, kind="ExternalInput").ap()
    x_oth = nc.dram_tensor("x_oth", [128, CH, TOWN], bf16, kind="ExternalInput").ap()
    wqk = nc.dram_tensor("wqk", [128, CH, 2 * DK], bf16, kind="ExternalInput").ap()
    wv = nc.dram_tensor("wv", [128, CH, DV], bf16, kind="ExternalInput").ap()
    bq = nc.dram_tensor("bq", [DK, 1], f32, kind="ExternalInput").ap()
    out = nc.dram_tensor("out", [T, DV + 1], bf16, kind="ExternalOutput").ap()

    with tile.TileContext(nc) as tc, ExitStack() as ctx:
        consts = ctx.enter_context(tc.tile_pool(name="consts", bufs=1))
        persist = ctx.enter_context(tc.tile_pool(name="persist", bufs=1))
        wpool = ctx.enter_context(tc.tile_pool(name="wpool", bufs=1))
        xpool = ctx.enter_context(tc.tile_pool(name="xpool", bufs=1))
        out_pool = ctx.enter_context(tc.tile_pool(name="out_pool", bufs=4))
        ps_pool = ctx.enter_context(tc.tile_pool(name="ps", bufs=2, space="PSUM"))
        sc_pool = ctx.enter_context(tc.tile_pool(name="sc", bufs=4, space="PSUM"))

        bq_sb = consts.tile([DK, 1], f32)
        nc.gpsimd.dma_start(out=bq_sb[:], in_=bq)

        qT_sb = persist.tile([128, T], bf16)  # [dk, q] all queries, q̂+bq
        kT_sb = persist.tile([128, TOWN], bf16)  # [dk, t-own]
        v_sb = persist.tile([128, TCH, DV + 2], bf16)  # [t-part, chunk, dv|1|pad]
        pT_sb = persist.tile([128, TCH, T], bf16)  # [t-part, chunk, q]

        nc.vector.memset(v_sb[:, :, DV : DV + 2], 1.0)

        xo_sb = xpool.tile([128, CH, TOWN], bf16)
        xt_sb = xpool.tile([128, CH, TOWN], bf16)
        wqk_sb = wpool.tile([128, CH, 2 * DK], bf16)
        wv_sb = wpool.tile([128, CH, DV], bf16)
        # Input DMAs ordered by first PE use, alternating rings so the q/k
        # phase is fed at chunk cadence and wv arrives just before v-proj.
        nc.scalar.dma_start(out=wqk_sb[:], in_=wqk)
        for c in range(CH):
            eng = nc.sync if c % 2 == 0 else nc.scalar
            eng.dma_start(out=xo_sb[:, c, :], in_=x_own[:, c, :])
        nc.sync.dma_start(out=wv_sb[:, 0:3, :], in_=wv[:, 0:3, :])
        nc.scalar.dma_start(out=wv_sb[:, 3:6, :], in_=wv[:, 3:6, :])
        nc.sync.dma_start(out=xt_sb[:, 0:3, :], in_=x_oth[:, 0:3, :])
        nc.scalar.dma_start(out=xt_sb[:, 3:6, :], in_=x_oth[:, 3:6, :])

        def emit_scores_t(t, qh):
            # scores^T for one own-key chunk, P^T = exp(scale*s) in bf16.
            # 512-col score tiles (1 PSUM bank each) so exps pipeline.
            for n0 in (0, 512):
                ps_s = sc_pool.tile([128, 512], f32, tag="sc")
                nc.tensor.matmul(
                    ps_s[:],
                    kT_sb[:, t * 128 : (t + 1) * 128],
                    qT_sb[:, qh * 1024 + n0 : qh * 1024 + n0 + 512],
                    start=True,
                    stop=True,
                )
                nc.scalar.activation(
                    pT_sb[:, t, qh * 1024 + n0 : qh * 1024 + n0 + 512],
                    ps_s[:],
                    mybir.ActivationFunctionType.Exp,
                    scale=SCALE,
                )

        def emit_v_t(t):
            # v-projection for one own-key chunk; c split so chunks 0:3
            # depend only on the first wv DMA
            ps_v = ps_pool.tile([128, 1024], f32, tag="ps")
            for c in range(CH):
                for n0, n1 in ((0, 512), (512, DV)):
                    nc.tensor.matmul(
                        ps_v[:, n0:n1],
                        xo_sb[:, c, t * 128 : (t + 1) * 128],
                        wv_sb[:, c, n0:n1],
                        start=(c == 0),
                        stop=(c == CH - 1),
                    )
            nc.vector.tensor_copy(v_sb[:, t, 0:DV], ps_v[:, 0:DV])

        def emit_out_qc(qc):
            # partial numerator + rowsum: out[qc] = sum_t P^T[t,qc].T @ [v|1];
            # copy+store each region as soon as its accumulation stops;
            # stores alternate rings (region0 sync, region1 scalar)
            ps_o = ps_pool.tile([128, 1024], f32, tag="ps")
            o_sb = out_pool.tile([128, DV + 1], bf16, tag="o")
            for reg, (n0, n1) in enumerate(((0, 512), (512, DV + 2))):
                for t in range(TCH):
                    nc.tensor.matmul(
                        ps_o[:, n0:n1],
                        pT_sb[:, t, qc * 128 : (qc + 1) * 128],
                        v_sb[:, t, n0:n1],
                        start=(t == 0),
                        stop=(t == TCH - 1),
                    )
                c1 = min(n1, DV + 1)
                nc.vector.tensor_copy(o_sb[:, n0:c1], ps_o[:, n0:c1])
                eng = nc.sync if reg == 0 else nc.scalar
                eng.dma_start(
                    out=out[qc * 128 : (qc + 1) * 128, n0:c1],
                    in_=o_sb[:, n0:c1],
                )

        # q own-half + k own, c-outer (PE consumes chunks as they stream)
        ps_q0 = ps_pool.tile([128, 1024], f32, tag="ps")
        ps_k = ps_pool.tile([128, 1024], f32, tag="ps")
        for c in range(CH):
            for n0 in (0, 512):
                nc.tensor.matmul(
                    ps_q0[:, n0 : n0 + 512],
                    wqk_sb[:, c, 0:DK],
                    xo_sb[:, c, n0 : n0 + 512],
                    start=(c == 0),
                    stop=(c == CH - 1),
                )
                nc.tensor.matmul(
                    ps_k[:, n0 : n0 + 512],
                    wqk_sb[:, c, DK : 2 * DK],
                    xo_sb[:, c, n0 : n0 + 512],
                    start=(c == 0),
                    stop=(c == CH - 1),
                )
        # qT = q̂+bq on scalar, split per 512 so scores t=0 unblocks early;
        # kT casts are per-chunk inside the loop below for the same reason
        for n0 in (0, 512):
            nc.scalar.activation(
                qT_sb[:, n0 : n0 + 512],
                ps_q0[:, n0 : n0 + 512],
                mybir.ActivationFunctionType.Identity,
                bias=bq_sb[:],
            )

        # scores for own queries interleaved with v-projection: the scalar
        # exps (~1.4us/chunk) hide under the v matmuls (~2.4us/chunk)
        for t in range(TCH):
            nc.vector.tensor_copy(
                kT_sb[:, t * 128 : (t + 1) * 128], ps_k[:, t * 128 : (t + 1) * 128]
            )
            emit_scores_t(t, 0)
            emit_v_t(t)

        # q other-half
        ps_q1 = ps_pool.tile([128, 1024], f32, tag="ps")
        for c in range(CH):
            for n0 in (0, 512):
                nc.tensor.matmul(
                    ps_q1[:, n0 : n0 + 512],
                    wqk_sb[:, c, 0:DK],
                    xt_sb[:, c, n0 : n0 + 512],
                    start=(c == 0),
                    stop=(c == CH - 1),
                )
        for n0 in (0, 512):
            nc.scalar.activation(
                qT_sb[:, TOWN + n0 : TOWN + n0 + 512],
                ps_q1[:, n0 : n0 + 512],
                mybir.ActivationFunctionType.Identity,
                bias=bq_sb[:],
            )

        # scores for other-half queries interleaved with out0: exps for
        # half 1 complete long before out1 consumes pT, with no PE stall
        for qc in range(8):
            emit_scores_t(qc, 1)
            emit_out_qc(qc)

        for qc in range(8, 16):
            emit_out_qc(qc)

    nc.compile()
    return nc


def _get_nc():
    if "nc" not in _CACHE:
        _CACHE["nc"] = _build()
    return _CACHE["nc"]


def _make_in_maps(x, Wq, bq, Wk, bk, Wv):
    bf16 = ml_dtypes.bfloat16
    wq = np.asarray(Wq, np.float32).astype(bf16).reshape(CH, 128, DK)
    wk = np.asarray(Wk, np.float32).astype(bf16).reshape(CH, 128, DK)
    base = {
        # partition-major packs: contiguous per-partition rows -> fat DMA
        # descriptors (wqk 3KB, wv 4.6KB/half, x 2KB rows)
        "wqk": np.ascontiguousarray(
            np.concatenate([wq, wk], axis=2).transpose(1, 0, 2)
        ),
        "wv": np.ascontiguousarray(
            np.asarray(Wv, np.float32).astype(bf16).reshape(CH, 128, DV).transpose(1, 0, 2)
        ),
        "bq": np.ascontiguousarray(np.asarray(bq, np.float32).reshape(DK, 1)),
    }
    in_maps = []
    for c in range(NCORES):
        b, h = c // 2, c % 2
        xb = x[b]  # [T, DIN]
        rot = np.concatenate([xb[h * TOWN :], xb[: h * TOWN]], axis=0)
        xT = rot.T.astype(bf16).reshape(CH, 128, T).transpose(1, 0, 2)  # [128,c,t]
        m = dict(base)
        m["x_own"] = np.ascontiguousarray(xT[:, :, 0:TOWN])
        m["x_oth"] = np.ascontiguousarray(xT[:, :, TOWN:T])
        in_maps.append(m)
    return in_maps


def kernel(x, Wq, bq, Wk, bk, Wv, bv):
    from concourse import bass_utils

    x = np.ascontiguousarray(np.asarray(x, dtype=np.float32))
    nc = _get_nc()
    in_maps = _make_in_maps(x, Wq, bq, Wk, bk, Wv)

    res = bass_utils.run_bass_kernel_spmd(nc, in_maps, core_ids=list(range(NCORES)))

    bv = np.asarray(bv, np.float32).reshape(1, DV)
    outp = np.empty((B, T, DV), dtype=np.float32)
    for b in range(B):
        p0 = res.results[2 * b]["out"]  # natural query order (h=0)
        p1 = res.results[2 * b + 1]["out"]  # rotated by TOWN (h=1)
        p1 = np.concatenate([p1[TOWN:], p1[:TOWN]], axis=0)
        s = p0.astype(np.float64) + p1.astype(np.float64)
        outp[b] = (s[:, 0:DV] / s[:, DV : DV + 1] + bv).astype(np.float32)
    return outp
